# revision 3
# baseline (speedup 1.0000x reference)
"""Trainium2 Bass kernel for InterpretableMultiHeadAttention (v2, bf16).

Problem (hardcoded): B=8, S=1024, D=1024, H=16, dk=64, fp32 I/O.
  V    = X @ W_v                          (shared values)
  Q_h  = X @ W_q[h], K_h = X @ W_k[h]
  S_h  = Q_h K_h^T / sqrt(dk) - 1e9 * causal_mask
  A_h  = softmax(S_h)
  Aavg = mean_h A_h                       (output 2)
  out  = (Aavg @ V) @ W_o                 (output 1)

Sharding: data-parallel over batch; one batch element per NeuronCore.
The padding mask input is all-ones by construction, so only the causal
mask is applied.

v2 design notes (vs the fp32r v1 at 403us):
  - ALL matmul operands are bf16. fp32r matmuls run the PE in
    fp32_mode=LOW_HIGH which draws enough power that the SW/thermal
    throttler clamps the PE clock to K=4/8 (observed: 205us at half
    rate mid-kernel). bf16 streams at the same 1 col/cycle but at far
    lower power, and enables automatic FWL for weight loads.
  - X^T and Aavg^T built with 16-bit XBAR DMA transposes (idle DMA
    engines) instead of PE transpose + PSUM round-trip.
  - Causal penalty added in PSUM by a PE matmul (ident^T @ pen) after
    each head pair's score matmuls, keeping DVE out of the hot loop.
  - Scores per (head, qb) go to one [128,1024] fp32 PSUM tile; exp is
    a single ACT instruction per head reading across both banks, with
    accum_out row-sums written per-head into a shared Z[:,h] column.
    One reciprocal per qb computes all 16 1/z columns.
  - Head-mean via single full-128-contraction diag matmuls
    accumulating into one 2-bank PSUM tile (the 64/64 row-tile split
    of v1 bought nothing: matmul cost is streamed columns only).
  - Phase 4 is software-pipelined one qb deep: iteration t emits
    means/copies for qb=t-1 before scores/exp for qb=t so the PE
    never stalls on the exp tail, and the V = X @ W_v groups are
    interleaved into phase 4's PE slack (ACT exp is the phase-4
    bottleneck at ~84us).
  - W_o is DMA'd + converted during phase 3/4, not at phase 6.

PSUM budget: ps_mm 2x[128,512] + ps_sc 2x[128,1024] + ps_av 1x[128,1024]
= 2 + 4 + 2 = 8 banks exactly.
"""

import os
from contextlib import ExitStack

import numpy as np

import concourse.bass as bass
import concourse.mybir as mybir
import concourse.tile as tile
from concourse import bacc
from concourse.bass_utils import run_bass_kernel_spmd
from concourse.masks import make_causal_mask, make_identity

F32 = mybir.dt.float32
BF16 = mybir.dt.bfloat16
AF = mybir.ActivationFunctionType

B, S, D, H, DK = 8, 1024, 1024, 16, 64
P = 128
SO = S // P  # 8 s-blocks
DO = D // P  # 8 d-blocks
NPAIR = H // 2  # 8 head pairs


def build_attention(ctx: ExitStack, tc: tile.TileContext, outs, ins):
    nc = tc.nc
    x, wq, wk, wv, wo = ins["x"], ins["wq"], ins["wk"], ins["wv"], ins["wo"]
    out, attn = outs["out"], outs["attn"]

    const = ctx.enter_context(tc.tile_pool(name="const", bufs=1))
    big = ctx.enter_context(tc.tile_pool(name="big", bufs=1))
    wstage = ctx.enter_context(tc.tile_pool(name="wstage", bufs=2))
    wqkb = ctx.enter_context(tc.tile_pool(name="wqkb", bufs=2))
    xload = ctx.enter_context(tc.tile_pool(name="xload", bufs=2))
    xbp = ctx.enter_context(tc.tile_pool(name="xbp", bufs=2))
    epool = ctx.enter_context(tc.tile_pool(name="epool", bufs=16))
    zpool = ctx.enter_context(tc.tile_pool(name="zpool", bufs=2))
    dgpool = ctx.enter_context(tc.tile_pool(name="dgpool", bufs=3))
    apool = ctx.enter_context(tc.tile_pool(name="apool", bufs=2))
    abpool = ctx.enter_context(tc.tile_pool(name="abpool", bufs=2))
    opool = ctx.enter_context(tc.tile_pool(name="opool", bufs=2))
    ps_mm = ctx.enter_context(tc.tile_pool(name="ps_mm", bufs=2, space="PSUM"))
    ps_sc = ctx.enter_context(tc.tile_pool(name="ps_sc", bufs=2, space="PSUM"))
    ps_av = ctx.enter_context(tc.tile_pool(name="ps_av", bufs=1, space="PSUM"))

    # ---- constants ----
    ident_b = const.tile([P, P], BF16)
    make_identity(nc, ident_b)
    # identity scaled by 1/H: diag matmul then also applies the head mean
    ident_hb = const.tile([P, P], BF16)
    nc.scalar.mul(ident_hb, ident_b, 1.0 / H)
    pen_b = const.tile([P, P], BF16)
    make_causal_mask(nc, pen_b, mask_val=-1e9)

    # ---- phase 1: X -> X^T (bf16) via ACT convert + XBAR DMA transpose ----
    XT = big.tile([P, DO, S], BF16, tag="xt")
    for sb in range(SO):
        x_in = xload.tile([P, D], F32, tag="x")
        nc.sync.dma_start(x_in, x[sb * P : (sb + 1) * P, :])
        xb = xbp.tile([P, D], BF16, tag="xb")
        nc.scalar.copy(xb, x_in)
        for db in range(DO):
            nc.sync.dma_start_transpose(
                XT[:, db, sb * P : (sb + 1) * P], xb[:, db * P : (db + 1) * P]
            )

    # ---- phase 3 (before V: PE starts sooner): Q^T / K^T per head pair ----
    QT = big.tile([P, NPAIR, S], BF16, tag="qt")
    KT = big.tile([P, NPAIR, S], BF16, tag="kt")
    for p in range(NPAIR):
        wqs = wstage.tile([P, DO, P], F32, tag="ws")
        for j in range(2):
            nc.sync.dma_start(
                wqs[:, :, j * DK : (j + 1) * DK],
                wq[2 * p + j].rearrange("(do di) k -> di do k", di=P),
            )
        wqb = wqkb.tile([P, DO, P], BF16, tag="wqb")
        nc.scalar.copy(wqb, wqs)
        wks = wstage.tile([P, DO, P], F32, tag="ws")
        for j in range(2):
            nc.sync.dma_start(
                wks[:, :, j * DK : (j + 1) * DK],
                wk[2 * p + j].rearrange("(do di) k -> di do k", di=P),
            )
        wkb = wqkb.tile([P, DO, P], BF16, tag="wkb")
        nc.scalar.copy(wkb, wks)
        for sc in range(2):
            psq = ps_mm.tile([P, 512], F32, tag="mm")
            for db in range(DO):
                nc.tensor.matmul(
                    psq,
                    lhsT=wqb[:, db, :],
                    rhs=XT[:, db, sc * 512 : (sc + 1) * 512],
                    start=(db == 0),
                    stop=(db == DO - 1),
                )
            nc.vector.tensor_copy(QT[:, p, sc * 512 : (sc + 1) * 512], psq)
            psk = ps_mm.tile([P, 512], F32, tag="mm")
            for db in range(DO):
                nc.tensor.matmul(
                    psk,
                    lhsT=wkb[:, db, :],
                    rhs=XT[:, db, sc * 512 : (sc + 1) * 512],
                    start=(db == 0),
                    stop=(db == DO - 1),
                )
            nc.vector.tensor_copy(KT[:, p, sc * 512 : (sc + 1) * 512], psk)

    # W_v and W_o loads + converts: DMA and ACT run under phase 3/4 compute.
    # ACT order stays Copy...Copy Exp...Exp (one activation-table switch).
    wv_b = big.tile([P, DO, D], BF16, tag="wv")
    for db in range(DO):
        ws = wstage.tile([P, D], F32, tag="ws")
        nc.sync.dma_start(ws, wv[db * P : (db + 1) * P, :])
        nc.scalar.copy(wv_b[:, db, :], ws)
    wo_b = big.tile([P, DO, D], BF16, tag="wo")
    for db in range(DO):
        ws = wstage.tile([P, D], F32, tag="ws")
        nc.sync.dma_start(ws, wo[db * P : (db + 1) * P, :])
        nc.scalar.copy(wo_b[:, db, :], ws)

    # ---- phase 4: scores -> softmax -> head-mean, pipelined one qb deep ----
    AT = big.tile([P, SO, S], BF16, tag="at")
    V = big.tile([P, SO, D], BF16, tag="v")

    def chunks_of(kv):
        return [(c0, min(512, kv - c0)) for c0 in range(0, kv, 512)]

    def emit_mean_and_copies(qb, kv, E_list, Z):
        # 1/z for all 16 heads in one instruction
        R = zpool.tile([P, H], F32, tag="r")
        nc.vector.reciprocal(R, Z)
        ps_a = ps_av.tile([P, 1024], F32, tag="av")
        for h in range(H):
            dg = dgpool.tile([P, P], BF16, tag="dg")
            nc.vector.tensor_mul(dg, ident_hb, R[:, h : h + 1].to_broadcast((P, P)))
            for c0, w in chunks_of(kv):
                nc.tensor.matmul(
                    ps_a[:, c0 : c0 + w],
                    lhsT=dg,
                    rhs=E_list[h][:, c0 : c0 + w],
                    start=(h == 0),
                    stop=(h == H - 1),
                    skip_group_check=True,
                )
        A_sb = apool.tile([P, 1024], F32, tag="asb")
        nc.vector.tensor_copy(A_sb[:, 0:kv], ps_a[:, 0:kv])
        A_b = abpool.tile([P, 1024], BF16, tag="ab")
        nc.vector.tensor_copy(A_b[:, 0:kv], ps_a[:, 0:kv])
        nc.sync.dma_start(attn[qb * P : (qb + 1) * P, 0:kv], A_sb[:, 0:kv])
        # Aavg^T blocks (and zero the never-written blocks above the diagonal)
        if qb < SO - 1:
            nc.gpsimd.memset(AT[:, qb + 1 :, qb * P : (qb + 1) * P], 0.0)
        for sblk in range(qb + 1):
            nc.sync.dma_start_transpose(
                AT[:, sblk, qb * P : (qb + 1) * P], A_b[:, sblk * P : (sblk + 1) * P]
            )

    pending = None
    for t in range(SO + 1):
        if pending is not None:
            emit_mean_and_copies(*pending)
            pending = None
        if t < SO:
            qb = t
            kv = (qb + 1) * P
            Z = zpool.tile([P, H], F32, tag="z")
            E_list = []
            for h2 in range(0, H, 2):
                ps_pair = []
                # both heads' score matmuls first: they run concurrently on
                # the 64x128 T0/T8 row-tiles (contraction partitions 0-63 /
                # 64-127); the full-array penalty matmuls follow.
                for h in (h2, h2 + 1):
                    hp, ho = h // 2, (h % 2) * DK
                    ps_s = ps_sc.tile([P, 1024], F32, tag="sc")
                    for c0, w in chunks_of(kv):
                        has_diag = c0 <= qb * P < c0 + w
                        nc.tensor.matmul(
                            ps_s[:, c0 : c0 + w],
                            lhsT=QT[ho : ho + DK, hp, qb * P : (qb + 1) * P],
                            rhs=KT[ho : ho + DK, hp, c0 : c0 + w],
                            start=True,
                            stop=not has_diag,
                            skip_group_check=True,
                        )
                    ps_pair.append(ps_s)
                for h in (h2, h2 + 1):
                    # causal penalty onto the diagonal block, in PSUM
                    nc.tensor.matmul(
                        ps_pair[h - h2][:, qb * P : kv],
                        lhsT=ident_b,
                        rhs=pen_b,
                        start=False,
                        stop=True,
                        skip_group_check=True,
                    )
                for h in (h2, h2 + 1):
                    E = epool.tile([P, 1024], BF16, tag="e")
                    nc.scalar.activation(
                        E[:, 0:kv],
                        ps_pair[h - h2][:, 0:kv],
                        AF.Exp,
                        scale=0.125,
                        accum_out=Z[:, h : h + 1],
                    )
                    E_list.append(E)
            pending = (qb, kv, E_list, Z)
            # interleave V = X @ W_v into phase 4's PE slack (ACT-bound)
            sb = t
            for ec in range(2):
                psv = ps_mm.tile([P, 512], F32, tag="mm")
                for db in range(DO):
                    nc.tensor.matmul(
                        psv,
                        lhsT=XT[:, db, sb * P : (sb + 1) * P],
                        rhs=wv_b[:, db, ec * 512 : (ec + 1) * 512],
                        start=(db == 0),
                        stop=(db == DO - 1),
                    )
                nc.vector.tensor_copy(V[:, sb, ec * 512 : (ec + 1) * 512], psv)

    # ---- phase 5: Hout^T = V^T @ Aavg^T ----
    HT = big.tile([P, DO, S], BF16, tag="ht")
    for qc in range(2):
        so_max = 4 if qc == 0 else 8
        for eb in range(DO):
            psh = ps_mm.tile([P, 512], F32, tag="mm")
            for so in range(so_max):
                nc.tensor.matmul(
                    psh,
                    lhsT=V[:, so, eb * P : (eb + 1) * P],
                    rhs=AT[:, so, qc * 512 : (qc + 1) * 512],
                    start=(so == 0),
                    stop=(so == so_max - 1),
                )
            nc.vector.tensor_copy(HT[:, eb, qc * 512 : (qc + 1) * 512], psh)

    # ---- phase 6: out = Hout @ W_o ----
    for qb in range(SO):
        for dc2 in range(2):
            pso = ps_mm.tile([P, 512], F32, tag="mm")
            for eb in range(DO):
                nc.tensor.matmul(
                    pso,
                    lhsT=HT[:, eb, qb * P : (qb + 1) * P],
                    rhs=wo_b[:, eb, dc2 * 512 : (dc2 + 1) * 512],
                    start=(eb == 0),
                    stop=(eb == DO - 1),
                )
            osb = opool.tile([P, 512], F32, tag="osb")
            nc.vector.tensor_copy(osb, pso)
            nc.sync.dma_start(
                out[qb * P : (qb + 1) * P, dc2 * 512 : (dc2 + 1) * 512], osb
            )


_CACHED = {}


def build_module():
    if "nc" in _CACHED:
        return _CACHED["nc"]
    nc = bacc.Bacc(
        "TRN2",
        target_bir_lowering=False,
        debug=False,
        enable_asserts=False,
        num_devices=B,
    )
    ins = {
        "x": nc.dram_tensor("x", [S, D], F32, kind="ExternalInput").ap(),
        "wq": nc.dram_tensor("wq", [H, D, DK], F32, kind="ExternalInput").ap(),
        "wk": nc.dram_tensor("wk", [H, D, DK], F32, kind="ExternalInput").ap(),
        "wv": nc.dram_tensor("wv", [D, D], F32, kind="ExternalInput").ap(),
        "wo": nc.dram_tensor("wo", [D, D], F32, kind="ExternalInput").ap(),
    }
    outs = {
        "out": nc.dram_tensor("out", [S, D], F32, kind="ExternalOutput").ap(),
        "attn": nc.dram_tensor("attn", [S, S], F32, kind="ExternalOutput").ap(),
    }
    with tile.TileContext(nc) as tc, ExitStack() as ctx:
        build_attention(ctx, tc, outs, ins)
    nc.compile()
    _CACHED["nc"] = nc
    return nc


LAST_RESULTS = None


def kernel(inputs, mask, W_q, W_k, W_v, W_o, trace=False):
    global LAST_RESULTS
    nc = build_module()
    inputs = np.ascontiguousarray(inputs, dtype=np.float32)
    weights = {
        "wq": np.ascontiguousarray(W_q, dtype=np.float32),
        "wk": np.ascontiguousarray(W_k, dtype=np.float32),
        "wv": np.ascontiguousarray(W_v, dtype=np.float32),
        "wo": np.ascontiguousarray(W_o, dtype=np.float32),
    }
    in_maps = [{"x": inputs[b], **weights} for b in range(B)]
    res = run_bass_kernel_spmd(nc, in_maps, core_ids=list(range(B)), trace=trace)
    LAST_RESULTS = res
    output = np.stack([res.results[b]["out"] for b in range(B)])
    attn_avg = np.stack([res.results[b]["attn"] for b in range(B)])
    return output, attn_avg
